# revision 1
# baseline (speedup 1.0000x reference)
"""MoE top-2 routing kernel for Trainium2, 8 NeuronCores, batch-sharded.

Math (per token): logits = x@gate_W + gate_b; top-2 + softmax -> comb[B,E];
h = relu(x@W1[e]+b1[e]); y = h@W2[e]+b2[e]; out = sum_e comb[:,e]*y_e.

Implementation: dense all-expert formulation per core (B_local=8192).
 - gating in exact fp32 on the PE (top-2 selection is order-sensitive),
 - expert MLP in float32r (TF32-class, ~1e-4 rel err) on the PE,
 - top-2/softmax/combine via small PE transposes + DVE/ACT elementwise ops.
Host side only reshapes/shards: x is transposed to xT[D+1, B] (ones row
appended so biases ride the matmul), weights are flattened/augmented.
"""

import sys
import numpy as np

for _p in ("/opt/trn_rl_repo", "/root/.axon_site/_ro/trn_rl_repo"):
    if _p not in sys.path:
        sys.path.append(_p)

import concourse.bass as bass
import concourse.tile as tile
from concourse import bacc, mybir
from concourse.bass_utils import run_bass_kernel_spmd

F32 = mybir.dt.float32
F32R = mybir.dt.float32r
ALU = mybir.AluOpType
ACTF = mybir.ActivationFunctionType

NCORES = 8
B, D, E, H, O = 65536, 784, 16, 64, 10
BL = B // NCORES            # 8192 tokens per core
DP = D + 1                  # 785: ones row appended for bias
EH = E * H                  # 1024
CH = 512                    # tokens per chunk
NCHUNK = BL // CH           # 16
# contraction chunks over DP: six of 128 plus one of 17
KCH = [(i * 128, 128) for i in range(6)] + [(768, DP - 768)]
NK = len(KCH)
NH = EH // 128              # 8 h-col chunks of 128

_CACHED = {}


def _build_program(loop_reps=1):
    nc = bacc.Bacc("TRN2", target_bir_lowering=False, debug=False,
                   num_devices=NCORES)
    xA_d = nc.dram_tensor("xA", [NCHUNK, 128, 6 * CH], F32, kind="ExternalInput").ap()
    xB_d = nc.dram_tensor("xB", [NCHUNK, DP - 768, CH], F32, kind="ExternalInput").ap()
    Wg_d = nc.dram_tensor("Wg", [DP, E], F32, kind="ExternalInput").ap()
    W1_d = nc.dram_tensor("W1a", [DP, EH], F32R, kind="ExternalInput").ap()
    W2_d = nc.dram_tensor("W2a", [EH + E, O], F32R, kind="ExternalInput").ap()
    SEL_d = nc.dram_tensor("SEL", [E, EH], F32R, kind="ExternalInput").ap()
    I16_d = nc.dram_tensor("I16", [E, E], F32, kind="ExternalInput").ap()
    I128_d = nc.dram_tensor("I128", [128, 128], F32, kind="ExternalInput").ap()
    out_d = nc.dram_tensor("out", [O, BL], F32, kind="ExternalOutput").ap()

    with tile.TileContext(nc) as tc:
        import contextlib
        with contextlib.ExitStack() as ctx:
            wp = ctx.enter_context(tc.tile_pool(name="weights", bufs=1))
            xp = ctx.enter_context(tc.tile_pool(name="xtiles", bufs=2))
            sp = ctx.enter_context(tc.tile_pool(name="work", bufs=2))
            ps_a = ctx.enter_context(tc.tile_pool(name="ps_a", bufs=1, space="PSUM"))
            ps_b = ctx.enter_context(tc.tile_pool(name="ps_b", bufs=1, space="PSUM"))
            ps_c = ctx.enter_context(tc.tile_pool(name="ps_c", bufs=1, space="PSUM"))
            ps_h = ctx.enter_context(tc.tile_pool(name="ps_h", bufs=3, space="PSUM"))
            ps_e = ctx.enter_context(tc.tile_pool(name="ps_e", bufs=2, space="PSUM"))

            # ---- load weights/constants once ----
            Wg_t, W1_t = [], []
            for k, (s, sz) in enumerate(KCH):
                wg = wp.tile([sz, E], F32, tag=f"wg{k}")
                nc.sync.dma_start(wg[:], Wg_d[s:s + sz, :])
                Wg_t.append(wg)
                w1 = wp.tile([sz, EH], F32R, tag=f"w1{k}")
                nc.sync.dma_start(w1[:], W1_d[s:s + sz, :])
                W1_t.append(w1)
            W2_t = []
            for n in range(NH):
                w2 = wp.tile([128, O], F32R, tag=f"w2{n}")
                nc.sync.dma_start(w2[:], W2_d[n * 128:(n + 1) * 128, :])
                W2_t.append(w2)
            W2b = wp.tile([E, O], F32R, tag="w2b")
            nc.sync.dma_start(W2b[:], W2_d[EH:EH + E, :])
            SEL_t = wp.tile([E, EH], F32R, tag="sel")
            nc.sync.dma_start(SEL_t[:], SEL_d[:])
            I16_t = wp.tile([E, E], F32, tag="i16")
            nc.sync.dma_start(I16_t[:], I16_d[:])
            I128_t = wp.tile([128, 128], F32, tag="i128")
            nc.sync.dma_start(I128_t[:], I128_d[:])

            def body(rep):
                for c in range(NCHUNK):
                    col0 = c * CH
                    # ---- stream xT chunk (contiguous tiled layout) ----
                    tA = xp.tile([128, 6 * CH], F32, tag="tA")
                    nc.sync.dma_start(tA[:], xA_d[c])
                    tB = xp.tile([DP - 768, CH], F32, tag="tB")
                    nc.sync.dma_start(tB[:], xB_d[c])
                    trA = xp.tile([128, 6 * CH], F32R, tag="trA")
                    nc.vector.tensor_copy(trA[:], tA[:])
                    trB = xp.tile([DP - 768, CH], F32R, tag="trB")
                    nc.vector.tensor_copy(trB[:], tB[:])
                    xt = [tA[:, k * CH:(k + 1) * CH] for k in range(6)] + [tB[:]]
                    xtr = [trA[:, k * CH:(k + 1) * CH] for k in range(6)] + [trB[:]]

                    # ---- gating: logitsT [16, CH] in fp32 ----
                    pg = ps_a.tile([E, CH], F32, tag="pa")
                    for k in range(NK):
                        nc.tensor.matmul(pg[:], Wg_t[k][:], xt[k],
                                         start=(k == 0), stop=(k == NK - 1))
                    lgT = sp.tile([E, CH], F32, tag="lgT")
                    nc.vector.tensor_copy(lgT[:], pg[:])
                    # transpose to [128, 4*16] via matmul with I16
                    pl = ps_b.tile([128, 4 * E], F32, tag="pb")
                    for j in range(4):
                        nc.tensor.matmul(pl[:, j * E:(j + 1) * E],
                                         lgT[:, j * 128:(j + 1) * 128],
                                         I16_t[:], start=True, stop=True)
                    lg = sp.tile([128, 4 * E], F32, tag="lg")
                    nc.vector.tensor_copy(lg[:], pl[:])

                    # ---- top-2 + softmax weights -> comb [128, 4, 16] ----
                    lg3 = lg[:].rearrange("p (a e) -> p a e", e=E)
                    m1 = sp.tile([128, 4], F32, tag="m1")
                    nc.vector.tensor_reduce(m1[:], lg3, axis=mybir.AxisListType.X,
                                            op=ALU.max)
                    m1b = m1[:].broadcast_to([128, 4, E])
                    ind1 = sp.tile([128, 4 * E], F32, tag="ind1")
                    ind1_3 = ind1[:].rearrange("p (a e) -> p a e", e=E)
                    nc.vector.tensor_tensor(ind1_3, lg3, m1b, op=ALU.is_equal)
                    msk = sp.tile([128, 4 * E], F32, tag="msk")
                    msk3 = msk[:].rearrange("p (a e) -> p a e", e=E)
                    nc.vector.scalar_tensor_tensor(msk3, ind1_3, -1e30, lg3,
                                                   op0=ALU.mult, op1=ALU.add)
                    m2 = sp.tile([128, 4], F32, tag="m2")
                    nc.vector.tensor_reduce(m2[:], msk3, axis=mybir.AxisListType.X,
                                            op=ALU.max)
                    m2b = m2[:].broadcast_to([128, 4, E])
                    ind2 = sp.tile([128, 4 * E], F32, tag="ind2")
                    ind2_3 = ind2[:].rearrange("p (a e) -> p a e", e=E)
                    nc.vector.tensor_tensor(ind2_3, msk3, m2b, op=ALU.is_equal)
                    dd = sp.tile([128, 4], F32, tag="dd")
                    nc.vector.tensor_tensor(dd[:], m2[:], m1[:], op=ALU.subtract)
                    w2s = sp.tile([128, 4], F32, tag="w2s")
                    nc.scalar.activation(w2s[:], dd[:], ACTF.Sigmoid)
                    w1s = sp.tile([128, 4], F32, tag="w1s")
                    nc.vector.tensor_scalar(w1s[:], w2s[:], -1.0, 1.0,
                                            op0=ALU.mult, op1=ALU.add)
                    w1b = w1s[:].broadcast_to([128, 4, E])
                    w2b_ = w2s[:].broadcast_to([128, 4, E])
                    comb = sp.tile([128, 4 * E], F32, tag="comb")
                    comb3 = comb[:].rearrange("p (a e) -> p a e", e=E)
                    nc.vector.tensor_tensor(comb3, ind1_3, w1b, op=ALU.mult)
                    c2 = sp.tile([128, 4 * E], F32, tag="c2")
                    c2_3 = c2[:].rearrange("p (a e) -> p a e", e=E)
                    nc.vector.tensor_tensor(c2_3, ind2_3, w2b_, op=ALU.mult)
                    nc.vector.tensor_tensor(comb[:], comb[:], c2[:], op=ALU.add)

                    # ---- combT [16, CH] (f32r) via matmul with I128 ----
                    pcT = ps_c.tile([E, CH], F32, tag="pcT")
                    for j in range(4):
                        nc.tensor.matmul(pcT[:, j * 128:(j + 1) * 128],
                                         comb[:, j * E:(j + 1) * E],
                                         I128_t[:], start=True, stop=True)
                    cT = sp.tile([E, CH], F32R, tag="cT")
                    nc.vector.tensor_copy(cT[:], pcT[:])

                    # ---- expert MLP (f32r) + weighted combine ----
                    po_full = ps_a.tile([E, CH], F32, tag="pa")
                    po = po_full[:O, :]
                    for n in range(NH):
                        ph = ps_h.tile([128, CH], F32, tag="ph")
                        for k in range(NK):
                            nc.tensor.matmul(
                                ph[:], W1_t[k][:, n * 128:(n + 1) * 128],
                                xtr[k], start=(k == 0), stop=(k == NK - 1))
                        pce = ps_e.tile([128, CH], F32, tag="pce")
                        nc.tensor.matmul(pce[:], SEL_t[:, n * 128:(n + 1) * 128],
                                         cT[:], start=True, stop=True)
                        hsb = sp.tile([128, CH], F32, tag="hsb")
                        nc.scalar.activation(hsb[:], ph[:], ACTF.Relu)
                        g = sp.tile([128, CH], F32R, tag="g")
                        nc.vector.tensor_tensor(g[:], hsb[:], pce[:], op=ALU.mult)
                        nc.tensor.matmul(po[:], W2_t[n][:], g[:],
                                         start=(n == 0), stop=False)
                    nc.tensor.matmul(po[:], W2b[:], cT[:], start=False, stop=True)

                    # ---- store transposed output [10, CH]; host untransposes ----
                    osb = sp.tile([O, CH], F32, tag="osb")
                    nc.vector.tensor_copy(osb[:], po[:])
                    nc.sync.dma_start(out_d[:, col0:col0 + CH], osb[:])

            if loop_reps > 1:
                with tc.For_i(0, loop_reps, 1) as _i:
                    body(_i)
            else:
                body(0)

    nc.compile()
    return nc


def _host_prep(x, gate_W, gate_b, W1, b1, W2, b2):
    x = np.asarray(x, np.float32)
    # xA[core, chunk, p, k*CH+j] = x[core*BL + chunk*CH + j, k*128+p], k<6
    xA = np.ascontiguousarray(
        x[:, :768].reshape(NCORES, NCHUNK, CH, 6, 128).transpose(0, 1, 4, 3, 2)
    ).reshape(NCORES, NCHUNK, 128, 6 * CH)
    # xB: d in [768,784) plus ones row (bias)
    xB = np.empty((NCORES, NCHUNK, DP - 768, CH), np.float32)
    xB[:, :, :D - 768, :] = x[:, 768:].reshape(
        NCORES, NCHUNK, CH, D - 768).transpose(0, 1, 3, 2)
    xB[:, :, D - 768:, :] = 1.0
    Wg = np.concatenate([np.asarray(gate_W, np.float32),
                         np.asarray(gate_b, np.float32)[None, :]], 0)
    W1f = np.asarray(W1, np.float32).transpose(1, 0, 2).reshape(D, EH)
    W1a = np.concatenate([W1f, np.asarray(b1, np.float32).reshape(1, EH)], 0)
    W2a = np.concatenate([np.asarray(W2, np.float32).reshape(EH, O),
                          np.asarray(b2, np.float32)], 0)
    SEL = np.zeros((E, EH), np.float32)
    for cidx in range(EH):
        SEL[cidx // H, cidx] = 1.0
    consts = {
        "Wg": Wg, "W1a": W1a, "W2a": W2a, "SEL": SEL,
        "I16": np.eye(E, dtype=np.float32),
        "I128": np.eye(128, dtype=np.float32),
    }
    return xA, xB, consts


def kernel(x, gate_W, gate_b, W1, b1, W2, b2, _loop_reps=1):
    if _loop_reps not in _CACHED:
        _CACHED[_loop_reps] = _build_program(_loop_reps)
    nc = _CACHED[_loop_reps]
    xA, xB, consts = _host_prep(x, gate_W, gate_b, W1, b1, W2, b2)
    in_maps = []
    for cidx in range(NCORES):
        m = dict(consts)
        m["xA"] = xA[cidx]
        m["xB"] = np.ascontiguousarray(xB[cidx])
        in_maps.append(m)
    res = run_bass_kernel_spmd(nc, in_maps, list(range(NCORES)))
    outT = np.concatenate([res.results[i]["out"] for i in range(NCORES)], 1)
    return np.ascontiguousarray(outT.T).astype(np.float32)



# revision 10
# speedup vs baseline: 15.5862x; 15.5862x over previous
"""MoE top-2 routing kernel for Trainium2, 8 NeuronCores, batch-sharded.

Math (per token): logits = x@gate_W + gate_b; top-2 + softmax -> comb[B,E];
h = relu(x@W1[e]+b1[e]); y = h@W2[e]+b2[e]; out = sum_e comb[:,e]*y_e.

Implementation: dense all-expert formulation per core (B_local=8192).
 - gating in exact fp32 on the PE (top-2 selection is order-sensitive),
 - expert MLP in bf16 (~1e-3 rel err, tolerance is 2e-2),
 - top-2/softmax/combine via small PE transposes + DVE/ACT elementwise ops.
DMA structure tuned for this environment: x is staged once into an
internal-DRAM scratch; the steady-state loop issues only 5 large DMAs per
iteration (4x 6.3MB x super-chunks + 1 output write).
"""

import sys
import numpy as np

for _p in ("/opt/trn_rl_repo", "/root/.axon_site/_ro/trn_rl_repo"):
    if _p not in sys.path:
        sys.path.append(_p)

import concourse.bass as bass
import concourse.tile as tile
from concourse import bacc, mybir
from concourse.bass_utils import run_bass_kernel_spmd

F32 = mybir.dt.float32
F32R = mybir.dt.float32r
BF16 = mybir.dt.bfloat16
ALU = mybir.AluOpType
ACTF = mybir.ActivationFunctionType

NCORES = 8
B, D, E, H, O = 65536, 784, 16, 64, 10
BL = B // NCORES            # 8192 tokens per core
DP = D + 1                  # 785: ones row appended for bias
EH = E * H                  # 1024
CH = 512                    # tokens per chunk (PSUM bank width in fp32)
NCHUNK = BL // CH           # 16
CPS = 2                     # chunks per super-chunk (one DMA)
NSUP = NCHUNK // CPS        # 4 super-chunks
SUPW = 6 * CPS * CH         # 12288 columns per super-chunk tile
NH = EH // 128              # 8 h-col chunks of 128

# staging x into device-DRAM scratch before the timed loop (off: external
# input reads measure at ~290GB/s/core here, and the staging write->read
# round trip corrupts a handful of rows on hardware)
STAGE = False
# single DRAM->DRAM staging DMA (False: bounce through SBUF)
STAGE_D2D = True

_CACHED = {}


def _build_program(loop_reps=1):
    nc = bacc.Bacc("TRN2", target_bir_lowering=False, debug=False,
                   num_devices=NCORES)
    xA_d = nc.dram_tensor("xA", [NSUP, 128, SUPW], F32, kind="ExternalInput").ap()
    xB_d = nc.dram_tensor("xB", [DP - 768, BL], F32, kind="ExternalInput").ap()
    Wg_d = nc.dram_tensor("Wg", [DP, E], F32, kind="ExternalInput").ap()
    W1_d = nc.dram_tensor("W1a", [DP, EH], BF16, kind="ExternalInput").ap()
    W2_d = nc.dram_tensor("W2a", [EH + E, O], BF16, kind="ExternalInput").ap()
    SEL_d = nc.dram_tensor("SEL", [E, EH], BF16, kind="ExternalInput").ap()
    I16_d = nc.dram_tensor("I16", [E, E], F32, kind="ExternalInput").ap()
    I128_d = nc.dram_tensor("I128", [128, 128], F32, kind="ExternalInput").ap()
    out_d = nc.dram_tensor("out", [O, BL], F32, kind="ExternalOutput").ap()

    with tile.TileContext(nc) as tc:
        import contextlib
        with contextlib.ExitStack() as ctx:
            wp = ctx.enter_context(tc.tile_pool(name="weights", bufs=1))
            xp = ctx.enter_context(tc.tile_pool(name="xtiles", bufs=2))
            sp = ctx.enter_context(tc.tile_pool(name="work", bufs=2))
            dpool = ctx.enter_context(tc.tile_pool(name="xhbm", bufs=1, space="DRAM"))
            ps_a = ctx.enter_context(tc.tile_pool(name="ps_a", bufs=1, space="PSUM"))
            ps_b = ctx.enter_context(tc.tile_pool(name="ps_b", bufs=1, space="PSUM"))
            ps_c = ctx.enter_context(tc.tile_pool(name="ps_c", bufs=1, space="PSUM"))
            ps_h = ctx.enter_context(tc.tile_pool(name="ps_h", bufs=3, space="PSUM"))
            ps_e = ctx.enter_context(tc.tile_pool(name="ps_e", bufs=2, space="PSUM"))

            # ---- load weights/constants once ----
            KCH = [(i * 128, 128) for i in range(6)]
            Wg_t, W1_t = [], []
            for k, (s, sz) in enumerate(KCH):
                wg = wp.tile([sz, E], F32, tag=f"wg{k}")
                nc.sync.dma_start(wg[:], Wg_d[s:s + sz, :])
                Wg_t.append(wg)
                w1 = wp.tile([sz, EH], BF16, tag=f"w1{k}")
                nc.sync.dma_start(w1[:], W1_d[s:s + sz, :])
                W1_t.append(w1)
            wgB = wp.tile([DP - 768, E], F32, tag="wgB")
            nc.sync.dma_start(wgB[:], Wg_d[768:DP, :])
            Wg_t.append(wgB)
            w1B = wp.tile([DP - 768, EH], BF16, tag="w1B")
            nc.sync.dma_start(w1B[:], W1_d[768:DP, :])
            W1_t.append(w1B)
            W2_t = []
            for n in range(NH):
                w2 = wp.tile([128, O], BF16, tag=f"w2{n}")
                nc.sync.dma_start(w2[:], W2_d[n * 128:(n + 1) * 128, :])
                W2_t.append(w2)
            W2b = wp.tile([E, O], BF16, tag="w2b")
            nc.sync.dma_start(W2b[:], W2_d[EH:EH + E, :])
            SEL_t = wp.tile([E, EH], BF16, tag="sel")
            nc.sync.dma_start(SEL_t[:], SEL_d[:])
            I16_t = wp.tile([E, E], F32, tag="i16")
            nc.sync.dma_start(I16_t[:], I16_d[:])
            I128_t = wp.tile([128, 128], F32, tag="i128")
            nc.sync.dma_start(I128_t[:], I128_d[:])

            # xB (rows 768..783 + ones row) for the whole core, SBUF-resident
            xBall = wp.tile([DP - 768, BL], F32, tag="xBall")
            nc.sync.dma_start(xBall[:], xB_d[:])
            xBmlp = wp.tile([DP - 768, BL], BF16, tag="xBmlp")
            nc.vector.tensor_copy(xBmlp[:], xBall[:])

            # output accumulator for the whole core, SBUF-resident
            osb_all = wp.tile([O, BL], F32, tag="osb_all")

            # ---- stage x into device-HBM scratch once ----
            if STAGE:
                xAi = dpool.tile([NSUP, 128, SUPW], F32, tag="xAi")
                if STAGE_D2D:
                    nc.sync.dma_start(xAi[:], xA_d[:])
                else:
                    for s in range(NSUP):
                        stg = xp.tile([128, SUPW], F32, tag="tS")
                        nc.sync.dma_start(stg[:], xA_d[s])
                        nc.sync.dma_start(xAi[s], stg[:])
                x_src = xAi
            else:
                x_src = xA_d
            out_i = dpool.tile([O, BL], F32, tag="out_i")

            def chunk_body(tS, trS, cc, col0):
                xt = [tS[:, (cc * 6 + k) * CH:(cc * 6 + k + 1) * CH]
                      for k in range(6)] + [xBall[:, col0:col0 + CH]]
                xtr = [trS[:, (cc * 6 + k) * CH:(cc * 6 + k + 1) * CH]
                       for k in range(6)] + [xBmlp[:, col0:col0 + CH]]

                # ---- gating: logitsT [16, CH] in fp32 ----
                pg = ps_a.tile([E, CH], F32, tag="pa")
                for k in range(7):
                    nc.tensor.matmul(pg[:], Wg_t[k][:], xt[k],
                                     start=(k == 0), stop=(k == 6))
                lgT = sp.tile([E, CH], F32, tag="lgT")
                nc.vector.tensor_copy(lgT[:], pg[:])
                # transpose to [128, 4*16] via matmul with I16
                pl = ps_b.tile([128, 4 * E], F32, tag="pb")
                for j in range(4):
                    nc.tensor.matmul(pl[:, j * E:(j + 1) * E],
                                     lgT[:, j * 128:(j + 1) * 128],
                                     I16_t[:], start=True, stop=True)
                lg = sp.tile([128, 4 * E], F32, tag="lg")
                nc.vector.tensor_copy(lg[:], pl[:])

                # ---- top-2 + softmax weights -> comb [128, 4, 16] ----
                lg3 = lg[:].rearrange("p (a e) -> p a e", e=E)
                m1 = sp.tile([128, 4], F32, tag="m1")
                nc.vector.tensor_reduce(m1[:], lg3, axis=mybir.AxisListType.X,
                                        op=ALU.max)
                m1b = m1[:].broadcast_to([128, 4, E])
                ind1 = sp.tile([128, 4 * E], F32, tag="ind1")
                ind1_3 = ind1[:].rearrange("p (a e) -> p a e", e=E)
                nc.vector.tensor_tensor(ind1_3, lg3, m1b, op=ALU.is_equal)
                msk = sp.tile([128, 4 * E], F32, tag="msk")
                msk3 = msk[:].rearrange("p (a e) -> p a e", e=E)
                nc.vector.scalar_tensor_tensor(msk3, ind1_3, -1e30, lg3,
                                               op0=ALU.mult, op1=ALU.add)
                m2 = sp.tile([128, 4], F32, tag="m2")
                nc.vector.tensor_reduce(m2[:], msk3, axis=mybir.AxisListType.X,
                                        op=ALU.max)
                m2b = m2[:].broadcast_to([128, 4, E])
                ind2 = sp.tile([128, 4 * E], F32, tag="ind2")
                ind2_3 = ind2[:].rearrange("p (a e) -> p a e", e=E)
                nc.vector.tensor_tensor(ind2_3, msk3, m2b, op=ALU.is_equal)
                dd = sp.tile([128, 4], F32, tag="dd")
                nc.vector.tensor_tensor(dd[:], m2[:], m1[:], op=ALU.subtract)
                w2s = sp.tile([128, 4], F32, tag="w2s")
                nc.scalar.activation(w2s[:], dd[:], ACTF.Sigmoid)
                w1s = sp.tile([128, 4], F32, tag="w1s")
                nc.vector.tensor_scalar(w1s[:], w2s[:], -1.0, 1.0,
                                        op0=ALU.mult, op1=ALU.add)
                w1b = w1s[:].broadcast_to([128, 4, E])
                w2b_ = w2s[:].broadcast_to([128, 4, E])
                comb = sp.tile([128, 4 * E], F32, tag="comb")
                comb3 = comb[:].rearrange("p (a e) -> p a e", e=E)
                nc.vector.tensor_tensor(comb3, ind1_3, w1b, op=ALU.mult)
                c2 = sp.tile([128, 4 * E], F32, tag="c2")
                c2_3 = c2[:].rearrange("p (a e) -> p a e", e=E)
                nc.vector.tensor_tensor(c2_3, ind2_3, w2b_, op=ALU.mult)
                nc.vector.tensor_tensor(comb[:], comb[:], c2[:], op=ALU.add)

                # ---- combT [16, CH] (f32r) via matmul with I128 ----
                pcT = ps_c.tile([E, CH], F32, tag="pcT")
                for j in range(4):
                    nc.tensor.matmul(pcT[:, j * 128:(j + 1) * 128],
                                     comb[:, j * E:(j + 1) * E],
                                     I128_t[:], start=True, stop=True)
                cT = sp.tile([E, CH], BF16, tag="cT")
                nc.vector.tensor_copy(cT[:], pcT[:])

                # ---- expert MLP (f32r) + weighted combine ----
                po_full = ps_a.tile([E, CH], F32, tag="pa")
                po = po_full[:O, :]
                for n in range(NH):
                    ph = ps_h.tile([128, CH], F32, tag="ph")
                    for k in range(7):
                        nc.tensor.matmul(
                            ph[:], W1_t[k][:, n * 128:(n + 1) * 128],
                            xtr[k], start=(k == 0), stop=(k == 6))
                    pce = ps_e.tile([128, CH], F32, tag="pce")
                    nc.tensor.matmul(pce[:], SEL_t[:, n * 128:(n + 1) * 128],
                                     cT[:], start=True, stop=True)
                    hsb = sp.tile([128, CH], F32, tag="hsb")
                    nc.scalar.activation(hsb[:], ph[:], ACTF.Relu)
                    g = sp.tile([128, CH], BF16, tag="g")
                    nc.vector.tensor_tensor(g[:], hsb[:], pce[:], op=ALU.mult)
                    nc.tensor.matmul(po[:], W2_t[n][:], g[:],
                                     start=(n == 0), stop=False)
                nc.tensor.matmul(po[:], W2b[:], cT[:], start=False, stop=True)

                # ---- chunk output into the SBUF-resident accumulator ----
                nc.vector.tensor_copy(osb_all[:, col0:col0 + CH], po[:])

            def body(rep):
                for s in range(NSUP):
                    tS = xp.tile([128, SUPW], F32, tag="tS")
                    nc.sync.dma_start(tS[:], x_src[s])
                    trS = xp.tile([128, SUPW], BF16, tag="trS")
                    nc.vector.tensor_copy(trS[:], tS[:])
                    for cc in range(CPS):
                        chunk_body(tS, trS, cc, (s * CPS + cc) * CH)
                # one output write per iteration (full inference result)
                nc.sync.dma_start(out_i[:], osb_all[:])

            if loop_reps > 1:
                with tc.For_i(0, loop_reps, 1) as _i:
                    body(_i)
            else:
                body(0)

            # ---- ship output off-device once ----
            nc.sync.dma_start(out_d[:], osb_all[:])

    nc.compile()
    return nc


def _host_prep(x, gate_W, gate_b, W1, b1, W2, b2):
    x = np.asarray(x, np.float32)
    # xA[core, s, p, (cc*6+k)*CH+j] = x[core*BL + (s*CPS+cc)*CH + j, k*128+p]
    xA = np.ascontiguousarray(
        x[:, :768].reshape(NCORES, NSUP, CPS, CH, 6, 128)
        .transpose(0, 1, 5, 2, 4, 3)
    ).reshape(NCORES, NSUP, 128, SUPW)
    # xB: d in [768,784) plus ones row (bias), all core columns
    xB = np.empty((NCORES, DP - 768, BL), np.float32)
    xB[:, :D - 768, :] = x[:, 768:].reshape(NCORES, BL, D - 768).transpose(0, 2, 1)
    xB[:, D - 768:, :] = 1.0
    Wg = np.concatenate([np.asarray(gate_W, np.float32),
                         np.asarray(gate_b, np.float32)[None, :]], 0)
    import ml_dtypes
    BF = ml_dtypes.bfloat16
    W1f = np.asarray(W1, np.float32).transpose(1, 0, 2).reshape(D, EH)
    W1a = np.concatenate([W1f, np.asarray(b1, np.float32).reshape(1, EH)],
                         0).astype(BF)
    W2a = np.concatenate([np.asarray(W2, np.float32).reshape(EH, O),
                          np.asarray(b2, np.float32)], 0).astype(BF)
    SEL = np.zeros((E, EH), BF)
    for cidx in range(EH):
        SEL[cidx // H, cidx] = 1.0
    consts = {
        "Wg": Wg, "W1a": W1a, "W2a": W2a, "SEL": SEL,
        "I16": np.eye(E, dtype=np.float32),
        "I128": np.eye(128, dtype=np.float32),
    }
    return xA, xB, consts


def kernel(x, gate_W, gate_b, W1, b1, W2, b2, _loop_reps=1):
    if _loop_reps not in _CACHED:
        _CACHED[_loop_reps] = _build_program(_loop_reps)
    nc = _CACHED[_loop_reps]
    xA, xB, consts = _host_prep(x, gate_W, gate_b, W1, b1, W2, b2)
    in_maps = []
    for cidx in range(NCORES):
        m = dict(consts)
        m["xA"] = xA[cidx]
        m["xB"] = np.ascontiguousarray(xB[cidx])
        in_maps.append(m)
    res = run_bass_kernel_spmd(nc, in_maps, list(range(NCORES)))
    outT = np.concatenate([res.results[i]["out"] for i in range(NCORES)], 1)
    return np.ascontiguousarray(outT.T).astype(np.float32)


# revision 13
# speedup vs baseline: 16.7744x; 1.0762x over previous
"""MoE top-2 routing kernel for Trainium2, 8 NeuronCores, batch-sharded.

Math (per token): logits = x@gate_W + gate_b; top-2 + softmax -> comb[B,E];
h = relu(x@W1[e]+b1[e]); y = h@W2[e]+b2[e]; out = sum_e comb[:,e]*y_e.

Implementation: dense all-expert formulation per core (B_local=8192).
 - gating in exact fp32 on the PE (top-2 selection is order-sensitive),
 - expert MLP in bf16 (~5e-3 rel err, tolerance is 2e-2),
 - relu fused into the expert-weight multiply on DVE (ACT instructions
   have ~4us latency each on this part and would dominate),
 - top-2/softmax/combine via small PE transposes + DVE elementwise ops.
Layout: x is padded host-side from 784 to 896 = 7*128 columns (ones row at
784 rides the gating/W1 bias; rest zeros), transposed to xT[896, B] tiles.
"""

import sys
import numpy as np

for _p in ("/opt/trn_rl_repo", "/root/.axon_site/_ro/trn_rl_repo"):
    if _p not in sys.path:
        sys.path.append(_p)

import concourse.bass as bass
import concourse.tile as tile
from concourse import bacc, mybir
from concourse.bass_utils import run_bass_kernel_spmd

F32 = mybir.dt.float32
F32R = mybir.dt.float32r
BF16 = mybir.dt.bfloat16
ALU = mybir.AluOpType
ACTF = mybir.ActivationFunctionType

NCORES = 8
B, D, E, H, O = 65536, 784, 16, 64, 10
BL = B // NCORES            # 8192 tokens per core
DP = 896                    # 784 + ones row (bias) + zero pad to 7*128
NK = DP // 128              # 7 contraction chunks of 128
EH = E * H                  # 1024
CH = 512                    # tokens per chunk (PSUM bank width in fp32)
NCHUNK = BL // CH           # 16
CPS = 2                     # chunks per super-chunk (one DMA)
NSUP = NCHUNK // CPS        # 8 super-chunks
SUPW = NK * CPS * CH        # 7168 columns per super-chunk tile
NH = EH // 128              # 8 h-col chunks of 128

_CACHED = {}


def _build_program(loop_reps=1):
    nc = bacc.Bacc("TRN2", target_bir_lowering=False, debug=False,
                   num_devices=NCORES)
    xA_d = nc.dram_tensor("xA", [NSUP, 128, SUPW], F32, kind="ExternalInput").ap()
    Wg_d = nc.dram_tensor("Wg", [DP, E], F32, kind="ExternalInput").ap()
    W1_d = nc.dram_tensor("W1a", [DP, EH], BF16, kind="ExternalInput").ap()
    W2_d = nc.dram_tensor("W2a", [EH + E, O], BF16, kind="ExternalInput").ap()
    SEL_d = nc.dram_tensor("SEL", [E, EH], BF16, kind="ExternalInput").ap()
    I128_d = nc.dram_tensor("I128", [128, 128], BF16, kind="ExternalInput").ap()
    out_d = nc.dram_tensor("out", [O, BL], F32, kind="ExternalOutput").ap()

    with tile.TileContext(nc) as tc:
        import contextlib
        with contextlib.ExitStack() as ctx:
            wp = ctx.enter_context(tc.tile_pool(name="weights", bufs=1))
            xp = ctx.enter_context(tc.tile_pool(name="xtiles", bufs=3))
            sp = ctx.enter_context(tc.tile_pool(name="work", bufs=2))
            dpool = ctx.enter_context(tc.tile_pool(name="ohbm", bufs=1, space="DRAM"))
            ps_a = ctx.enter_context(tc.tile_pool(name="ps_a", bufs=2, space="PSUM"))
            ps_b = ctx.enter_context(tc.tile_pool(name="ps_b", bufs=1, space="PSUM"))
            ps_c = ctx.enter_context(tc.tile_pool(name="ps_c", bufs=1, space="PSUM"))
            ps_h = ctx.enter_context(tc.tile_pool(name="ps_h", bufs=3, space="PSUM"))
            ps_e = ctx.enter_context(tc.tile_pool(name="ps_e", bufs=1, space="PSUM"))

            # ---- load weights/constants once ----
            Wg_t, W1_t = [], []
            for k in range(NK):
                wg = wp.tile([128, E], F32, tag=f"wg{k}")
                nc.sync.dma_start(wg[:], Wg_d[k * 128:(k + 1) * 128, :])
                Wg_t.append(wg)
                w1 = wp.tile([128, EH], BF16, tag=f"w1{k}")
                nc.sync.dma_start(w1[:], W1_d[k * 128:(k + 1) * 128, :])
                W1_t.append(w1)
            W2_t = []
            for n in range(NH):
                w2 = wp.tile([128, O], BF16, tag=f"w2{n}")
                nc.sync.dma_start(w2[:], W2_d[n * 128:(n + 1) * 128, :])
                W2_t.append(w2)
            W2b = wp.tile([E, O], BF16, tag="w2b")
            nc.sync.dma_start(W2b[:], W2_d[EH:EH + E, :])
            SEL_t = wp.tile([E, EH], BF16, tag="sel")
            nc.sync.dma_start(SEL_t[:], SEL_d[:])
            I128_t = wp.tile([128, 128], BF16, tag="i128")
            nc.sync.dma_start(I128_t[:], I128_d[:])

            # output accumulator for the whole core, SBUF-resident
            osb_all = wp.tile([O, BL], F32, tag="osb_all")
            out_i = dpool.tile([O, BL], F32, tag="out_i")

            def chunk_body(tS, trS, cc, col0):
                xt = [tS[:, (cc * NK + k) * CH:(cc * NK + k + 1) * CH]
                      for k in range(NK)]
                xtr = [trS[:, (cc * NK + k) * CH:(cc * NK + k + 1) * CH]
                       for k in range(NK)]

                # ---- gating, token-major: lg[tok, e] directly, exact fp32
                # (stationary = x slice [128d x 128tok], moving = Wg[128d,16];
                #  short 16-wide streams, ld_weights dominates but still ~3.5x
                #  cheaper than expert-major + PE transpose) ----
                pl = ps_b.tile([128, 4 * E], F32, tag="pb")
                for b in range(4):
                    for k in range(NK):
                        nc.tensor.matmul(
                            pl[:, b * E:(b + 1) * E],
                            xt[k][:, b * 128:(b + 1) * 128], Wg_t[k][:],
                            start=(k == 0), stop=(k == NK - 1))
                lg = sp.tile([128, 4 * E], F32, tag="lg")
                nc.vector.tensor_copy(lg[:], pl[:])

                # ---- top-2 + softmax weights -> comb [128, 4, 16] ----
                lg3 = lg[:].rearrange("p (a e) -> p a e", e=E)
                m1 = sp.tile([128, 4], F32, tag="m1")
                nc.vector.tensor_reduce(m1[:], lg3, axis=mybir.AxisListType.X,
                                        op=ALU.max)
                m1b = m1[:].broadcast_to([128, 4, E])
                ind1 = sp.tile([128, 4 * E], F32, tag="ind1")
                ind1_3 = ind1[:].rearrange("p (a e) -> p a e", e=E)
                nc.vector.tensor_tensor(ind1_3, lg3, m1b, op=ALU.is_equal)
                msk = sp.tile([128, 4 * E], F32, tag="msk")
                msk3 = msk[:].rearrange("p (a e) -> p a e", e=E)
                nc.vector.scalar_tensor_tensor(msk3, ind1_3, -1e30, lg3,
                                               op0=ALU.mult, op1=ALU.add)
                m2 = sp.tile([128, 4], F32, tag="m2")
                nc.vector.tensor_reduce(m2[:], msk3, axis=mybir.AxisListType.X,
                                        op=ALU.max)
                m2b = m2[:].broadcast_to([128, 4, E])
                ind2 = sp.tile([128, 4 * E], F32, tag="ind2")
                ind2_3 = ind2[:].rearrange("p (a e) -> p a e", e=E)
                nc.vector.tensor_tensor(ind2_3, msk3, m2b, op=ALU.is_equal)
                dd = sp.tile([128, 4], F32, tag="dd")
                nc.vector.tensor_tensor(dd[:], m2[:], m1[:], op=ALU.subtract)
                w2s = sp.tile([128, 4], F32, tag="w2s")
                nc.scalar.activation(w2s[:], dd[:], ACTF.Sigmoid)
                w1s = sp.tile([128, 4], F32, tag="w1s")
                nc.vector.tensor_scalar(w1s[:], w2s[:], -1.0, 1.0,
                                        op0=ALU.mult, op1=ALU.add)
                w1b = w1s[:].broadcast_to([128, 4, E])
                w2b_ = w2s[:].broadcast_to([128, 4, E])
                comb = sp.tile([128, 4 * E], BF16, tag="comb")
                comb3 = comb[:].rearrange("p (a e) -> p a e", e=E)
                nc.vector.tensor_tensor(comb3, ind1_3, w1b, op=ALU.mult)
                c2 = sp.tile([128, 4 * E], BF16, tag="c2")
                c2_3 = c2[:].rearrange("p (a e) -> p a e", e=E)
                nc.vector.tensor_tensor(c2_3, ind2_3, w2b_, op=ALU.mult)
                nc.vector.tensor_tensor(comb[:], comb[:], c2[:], op=ALU.add)

                # ---- combT [16, CH] (bf16) via matmul with I128 ----
                pcT = ps_c.tile([E, CH], F32, tag="pcT")
                for j in range(4):
                    nc.tensor.matmul(pcT[:, j * 128:(j + 1) * 128],
                                     comb[:, j * E:(j + 1) * E],
                                     I128_t[:], start=True, stop=True)
                cT = sp.tile([E, CH], BF16, tag="cT")
                nc.vector.tensor_copy(cT[:], pcT[:])

                # ---- expert MLP (bf16) + weighted combine ----
                po_full = ps_a.tile([E, CH], F32, tag="pa")
                po = po_full[:O, :]
                for n in range(NH):
                    ph = ps_h.tile([128, CH], F32, tag="ph")
                    for k in range(NK):
                        nc.tensor.matmul(
                            ph[:], W1_t[k][:, n * 128:(n + 1) * 128],
                            xtr[k], start=(k == 0), stop=(k == NK - 1))
                    pce = ps_e.tile([128, CH], F32, tag="pce")
                    nc.tensor.matmul(pce[:], SEL_t[:, n * 128:(n + 1) * 128],
                                     cT[:], start=True, stop=True)
                    # relu on DVE (ACT instructions cost ~4us each here), then
                    # expert-weight multiply; two ops since DVE may read only
                    # one PSUM operand per instruction
                    hsb = sp.tile([128, CH], F32, tag="hsb")
                    nc.vector.tensor_scalar_max(hsb[:], ph[:], 0.0)
                    g = sp.tile([128, CH], BF16, tag="g")
                    nc.vector.tensor_tensor(g[:], hsb[:], pce[:], op=ALU.mult)
                    nc.tensor.matmul(po[:], W2_t[n][:], g[:],
                                     start=(n == 0), stop=False)
                nc.tensor.matmul(po[:], W2b[:], cT[:], start=False, stop=True)

                # ---- chunk output into the SBUF-resident accumulator ----
                nc.vector.tensor_copy(osb_all[:, col0:col0 + CH], po[:])

            def body(rep):
                for s in range(NSUP):
                    tS = xp.tile([128, SUPW], F32, tag="tS")
                    nc.sync.dma_start(tS[:], xA_d[s])
                    trS = xp.tile([128, SUPW], BF16, tag="trS")
                    nc.vector.tensor_copy(trS[:], tS[:])
                    for cc in range(CPS):
                        chunk_body(tS, trS, cc, (s * CPS + cc) * CH)
                # one output write per iteration (full inference result)
                nc.sync.dma_start(out_i[:], osb_all[:])

            if loop_reps > 1:
                with tc.For_i(0, loop_reps, 1) as _i:
                    body(_i)
            else:
                body(0)

            # ---- ship output off-device once ----
            nc.sync.dma_start(out_d[:], osb_all[:])

    nc.compile()
    return nc


def _host_prep(x, gate_W, gate_b, W1, b1, W2, b2):
    import ml_dtypes
    BF = ml_dtypes.bfloat16
    x = np.asarray(x, np.float32)
    # pad x with a ones column (bias row) and zeros to DP=896 columns
    xpad = np.zeros((B, DP), np.float32)
    xpad[:, :D] = x
    xpad[:, D] = 1.0
    # xA[core, s, p, (cc*NK+k)*CH+j] = xpad[core*BL+(s*CPS+cc)*CH+j, k*128+p]
    xA = np.ascontiguousarray(
        xpad.reshape(NCORES, NSUP, CPS, CH, NK, 128).transpose(0, 1, 5, 2, 4, 3)
    ).reshape(NCORES, NSUP, 128, SUPW)
    Wg = np.zeros((DP, E), np.float32)
    Wg[:D] = np.asarray(gate_W, np.float32)
    Wg[D] = np.asarray(gate_b, np.float32)
    W1a = np.zeros((DP, EH), np.float32)
    W1a[:D] = np.asarray(W1, np.float32).transpose(1, 0, 2).reshape(D, EH)
    W1a[D] = np.asarray(b1, np.float32).reshape(EH)
    W2a = np.concatenate([np.asarray(W2, np.float32).reshape(EH, O),
                          np.asarray(b2, np.float32)], 0).astype(BF)
    SEL = np.zeros((E, EH), BF)
    for cidx in range(EH):
        SEL[cidx // H, cidx] = 1.0
    consts = {
        "Wg": Wg, "W1a": W1a.astype(BF), "W2a": W2a, "SEL": SEL,
        "I128": np.eye(128, dtype=BF),
    }
    return xA, consts


def kernel(x, gate_W, gate_b, W1, b1, W2, b2, _loop_reps=1):
    if _loop_reps not in _CACHED:
        _CACHED[_loop_reps] = _build_program(_loop_reps)
    nc = _CACHED[_loop_reps]
    xA, consts = _host_prep(x, gate_W, gate_b, W1, b1, W2, b2)
    in_maps = []
    for cidx in range(NCORES):
        m = dict(consts)
        m["xA"] = xA[cidx]
        in_maps.append(m)
    res = run_bass_kernel_spmd(nc, in_maps, list(range(NCORES)))
    outT = np.concatenate([res.results[i]["out"] for i in range(NCORES)], 1)
    return np.ascontiguousarray(outT.T).astype(np.float32)


# revision 14
# speedup vs baseline: 22.2740x; 1.3279x over previous
"""MoE top-2 routing kernel for Trainium2, 8 NeuronCores, batch-sharded.

Math (per token): logits = x@gate_W + gate_b; top-2 + softmax -> comb[B,E];
h = relu(x@W1[e]+b1[e]); y = h@W2[e]+b2[e]; out = sum_e comb[:,e]*y_e.

Implementation: dense all-expert formulation per core (B_local=8192).
 - gating in exact fp32 on the PE (top-2 selection is order-sensitive),
 - expert MLP in bf16 (~5e-3 rel err, tolerance is 2e-2),
 - relu fused into the expert-weight multiply on DVE (ACT instructions
   have ~4us latency each on this part and would dominate),
 - top-2/softmax/combine via small PE transposes + DVE elementwise ops.
Layout: x is padded host-side from 784 to 896 = 7*128 columns (ones row at
784 rides the gating/W1 bias; rest zeros), transposed to xT[896, B] tiles.
"""

import sys
import numpy as np

for _p in ("/opt/trn_rl_repo", "/root/.axon_site/_ro/trn_rl_repo"):
    if _p not in sys.path:
        sys.path.append(_p)

import concourse.bass as bass
import concourse.tile as tile
from concourse import bacc, mybir
from concourse.bass_utils import run_bass_kernel_spmd

F32 = mybir.dt.float32
F32R = mybir.dt.float32r
BF16 = mybir.dt.bfloat16
ALU = mybir.AluOpType
ACTF = mybir.ActivationFunctionType

NCORES = 8
B, D, E, H, O = 65536, 784, 16, 64, 10
BL = B // NCORES            # 8192 tokens per core
DP = 896                    # 784 + ones row (bias) + zero pad to 7*128
NK = DP // 128              # 7 contraction chunks of 128
EH = E * H                  # 1024
CH = 512                    # tokens per chunk (PSUM bank width in fp32)
NCHUNK = BL // CH           # 16
CPS = 2                     # chunks per super-chunk (one DMA)
NSUP = NCHUNK // CPS        # 8 super-chunks
SUPW = NK * CPS * CH        # 7168 columns per super-chunk tile
NH = EH // 128              # 8 h-col chunks of 128

_CACHED = {}


def _build_program(loop_reps=1):
    nc = bacc.Bacc("TRN2", target_bir_lowering=False, debug=False,
                   num_devices=NCORES)
    xA_d = nc.dram_tensor("xA", [NSUP, 128, SUPW], F32, kind="ExternalInput").ap()
    Wg_d = nc.dram_tensor("Wg", [DP, E], F32, kind="ExternalInput").ap()
    W1_d = nc.dram_tensor("W1a", [DP, EH], BF16, kind="ExternalInput").ap()
    W2_d = nc.dram_tensor("W2a", [EH + E, O], BF16, kind="ExternalInput").ap()
    SEL_d = nc.dram_tensor("SEL", [E, EH], BF16, kind="ExternalInput").ap()
    I16_d = nc.dram_tensor("I16", [E, E], F32, kind="ExternalInput").ap()
    I128_d = nc.dram_tensor("I128", [128, 128], BF16, kind="ExternalInput").ap()
    out_d = nc.dram_tensor("out", [O, BL], F32, kind="ExternalOutput").ap()

    with tile.TileContext(nc) as tc:
        import contextlib
        with contextlib.ExitStack() as ctx:
            wp = ctx.enter_context(tc.tile_pool(name="weights", bufs=1))
            xp = ctx.enter_context(tc.tile_pool(name="xtiles", bufs=3))
            sp = ctx.enter_context(tc.tile_pool(name="work", bufs=2))
            dpool = ctx.enter_context(tc.tile_pool(name="ohbm", bufs=1, space="DRAM"))
            ps_a = ctx.enter_context(tc.tile_pool(name="ps_a", bufs=1, space="PSUM"))
            ps_b = ctx.enter_context(tc.tile_pool(name="ps_b", bufs=1, space="PSUM"))
            ps_c = ctx.enter_context(tc.tile_pool(name="ps_c", bufs=1, space="PSUM"))
            ps_h = ctx.enter_context(tc.tile_pool(name="ps_h", bufs=2, space="PSUM"))
            ps_e = ctx.enter_context(tc.tile_pool(name="ps_e", bufs=1, space="PSUM"))
            ps_o = ctx.enter_context(tc.tile_pool(name="ps_o", bufs=2, space="PSUM"))

            # ---- load weights/constants once ----
            Wg_t, W1_t = [], []
            for k in range(NK):
                wg = wp.tile([128, E], F32, tag=f"wg{k}")
                nc.sync.dma_start(wg[:], Wg_d[k * 128:(k + 1) * 128, :])
                Wg_t.append(wg)
                w1 = wp.tile([128, EH], BF16, tag=f"w1{k}")
                nc.sync.dma_start(w1[:], W1_d[k * 128:(k + 1) * 128, :])
                W1_t.append(w1)
            W2_t = []
            for n in range(NH):
                w2 = wp.tile([128, O], BF16, tag=f"w2{n}")
                nc.sync.dma_start(w2[:], W2_d[n * 128:(n + 1) * 128, :])
                W2_t.append(w2)
            W2b = wp.tile([E, O], BF16, tag="w2b")
            nc.sync.dma_start(W2b[:], W2_d[EH:EH + E, :])
            SEL_t = wp.tile([E, EH], BF16, tag="sel")
            nc.sync.dma_start(SEL_t[:], SEL_d[:])
            I16_t = wp.tile([E, E], F32, tag="i16")
            nc.sync.dma_start(I16_t[:], I16_d[:])
            I128_t = wp.tile([128, 128], BF16, tag="i128")
            nc.sync.dma_start(I128_t[:], I128_d[:])

            # output accumulator for the whole core, SBUF-resident
            osb_all = wp.tile([O, BL], F32, tag="osb_all")
            out_i = dpool.tile([O, BL], F32, tag="out_i")

            def chunk_body(tS, trS, cc, col0):
                xt = [tS[:, (cc * NK + k) * CH:(cc * NK + k + 1) * CH]
                      for k in range(NK)]
                xtr = [trS[:, (cc * NK + k) * CH:(cc * NK + k + 1) * CH]
                       for k in range(NK)]

                # ---- gating: logitsT [16, CH] in fp32 ----
                pg = ps_a.tile([E, CH], F32, tag="pa")
                for k in range(NK):
                    nc.tensor.matmul(pg[:], Wg_t[k][:], xt[k],
                                     start=(k == 0), stop=(k == NK - 1))
                lgT = sp.tile([E, CH], F32, tag="lgT")
                nc.vector.tensor_copy(lgT[:], pg[:])
                # transpose to [128, 4*16] via matmul with I16
                pl = ps_b.tile([128, 4 * E], F32, tag="pb")
                for j in range(4):
                    nc.tensor.matmul(pl[:, j * E:(j + 1) * E],
                                     lgT[:, j * 128:(j + 1) * 128],
                                     I16_t[:], start=True, stop=True)
                lg = sp.tile([128, 4 * E], F32, tag="lg")
                nc.vector.tensor_copy(lg[:], pl[:])

                # ---- top-2 + softmax weights -> comb [128, 4, 16] ----
                lg3 = lg[:].rearrange("p (a e) -> p a e", e=E)
                m1 = sp.tile([128, 4], F32, tag="m1")
                nc.vector.tensor_reduce(m1[:], lg3, axis=mybir.AxisListType.X,
                                        op=ALU.max)
                m1b = m1[:].broadcast_to([128, 4, E])
                ind1 = sp.tile([128, 4 * E], F32, tag="ind1")
                ind1_3 = ind1[:].rearrange("p (a e) -> p a e", e=E)
                nc.vector.tensor_tensor(ind1_3, lg3, m1b, op=ALU.is_equal)
                msk = sp.tile([128, 4 * E], F32, tag="msk")
                msk3 = msk[:].rearrange("p (a e) -> p a e", e=E)
                nc.vector.scalar_tensor_tensor(msk3, ind1_3, -1e30, lg3,
                                               op0=ALU.mult, op1=ALU.add)
                m2 = sp.tile([128, 4], F32, tag="m2")
                nc.vector.tensor_reduce(m2[:], msk3, axis=mybir.AxisListType.X,
                                        op=ALU.max)
                m2b = m2[:].broadcast_to([128, 4, E])
                ind2 = sp.tile([128, 4 * E], F32, tag="ind2")
                ind2_3 = ind2[:].rearrange("p (a e) -> p a e", e=E)
                nc.vector.tensor_tensor(ind2_3, msk3, m2b, op=ALU.is_equal)
                dd = sp.tile([128, 4], F32, tag="dd")
                nc.vector.tensor_tensor(dd[:], m2[:], m1[:], op=ALU.subtract)
                w2s = sp.tile([128, 4], F32, tag="w2s")
                nc.scalar.activation(w2s[:], dd[:], ACTF.Sigmoid)
                w1s = sp.tile([128, 4], F32, tag="w1s")
                nc.vector.tensor_scalar(w1s[:], w2s[:], -1.0, 1.0,
                                        op0=ALU.mult, op1=ALU.add)
                w1b = w1s[:].broadcast_to([128, 4, E])
                w2b_ = w2s[:].broadcast_to([128, 4, E])
                comb = sp.tile([128, 4 * E], BF16, tag="comb")
                comb3 = comb[:].rearrange("p (a e) -> p a e", e=E)
                nc.vector.tensor_tensor(comb3, ind1_3, w1b, op=ALU.mult)
                c2 = sp.tile([128, 4 * E], BF16, tag="c2")
                c2_3 = c2[:].rearrange("p (a e) -> p a e", e=E)
                nc.vector.tensor_tensor(c2_3, ind2_3, w2b_, op=ALU.mult)
                nc.vector.tensor_tensor(comb[:], comb[:], c2[:], op=ALU.add)

                # ---- combT [16, CH] (bf16) via matmul with I128 ----
                pcT = ps_c.tile([E, CH], F32, tag="pcT")
                for j in range(4):
                    nc.tensor.matmul(pcT[:, j * 128:(j + 1) * 128],
                                     comb[:, j * E:(j + 1) * E],
                                     I128_t[:], start=True, stop=True)
                cT = sp.tile([E, CH], BF16, tag="cT")
                nc.vector.tensor_copy(cT[:], pcT[:])

                # ---- expert MLP (bf16) + weighted combine ----
                po_full = ps_o.tile([E, CH], F32, tag="po")
                po = po_full[:O, :]
                for n in range(NH):
                    ph = ps_h.tile([128, CH], F32, tag="ph")
                    for k in range(NK):
                        nc.tensor.matmul(
                            ph[:], W1_t[k][:, n * 128:(n + 1) * 128],
                            xtr[k], start=(k == 0), stop=(k == NK - 1))
                    pce = ps_e.tile([128, CH], F32, tag="pce")
                    nc.tensor.matmul(pce[:], SEL_t[:, n * 128:(n + 1) * 128],
                                     cT[:], start=True, stop=True)
                    # relu on DVE (ACT instructions cost ~4us each here), then
                    # expert-weight multiply; two ops since DVE may read only
                    # one PSUM operand per instruction
                    hsb = sp.tile([128, CH], F32, tag="hsb")
                    nc.vector.tensor_scalar_max(hsb[:], ph[:], 0.0)
                    g = sp.tile([128, CH], BF16, tag="g")
                    nc.vector.tensor_tensor(g[:], hsb[:], pce[:], op=ALU.mult)
                    nc.tensor.matmul(po[:], W2_t[n][:], g[:],
                                     start=(n == 0), stop=False)
                nc.tensor.matmul(po[:], W2b[:], cT[:], start=False, stop=True)

                # ---- chunk output into the SBUF-resident accumulator ----
                nc.vector.tensor_copy(osb_all[:, col0:col0 + CH], po[:])

            def body(rep):
                for s in range(NSUP):
                    tS = xp.tile([128, SUPW], F32, tag="tS")
                    nc.sync.dma_start(tS[:], xA_d[s])
                    trS = xp.tile([128, SUPW], BF16, tag="trS")
                    nc.vector.tensor_copy(trS[:], tS[:])
                    for cc in range(CPS):
                        chunk_body(tS, trS, cc, (s * CPS + cc) * CH)
                # one output write per iteration (full inference result)
                nc.sync.dma_start(out_i[:], osb_all[:])

            if loop_reps > 1:
                with tc.For_i(0, loop_reps, 1) as _i:
                    body(_i)
            else:
                body(0)

            # ---- ship output off-device once ----
            nc.sync.dma_start(out_d[:], osb_all[:])

    nc.compile()
    return nc


def _host_prep(x, gate_W, gate_b, W1, b1, W2, b2):
    import ml_dtypes
    BF = ml_dtypes.bfloat16
    x = np.asarray(x, np.float32)
    # pad x with a ones column (bias row) and zeros to DP=896 columns
    xpad = np.zeros((B, DP), np.float32)
    xpad[:, :D] = x
    xpad[:, D] = 1.0
    # xA[core, s, p, (cc*NK+k)*CH+j] = xpad[core*BL+(s*CPS+cc)*CH+j, k*128+p]
    xA = np.ascontiguousarray(
        xpad.reshape(NCORES, NSUP, CPS, CH, NK, 128).transpose(0, 1, 5, 2, 4, 3)
    ).reshape(NCORES, NSUP, 128, SUPW)
    Wg = np.zeros((DP, E), np.float32)
    Wg[:D] = np.asarray(gate_W, np.float32)
    Wg[D] = np.asarray(gate_b, np.float32)
    W1a = np.zeros((DP, EH), np.float32)
    W1a[:D] = np.asarray(W1, np.float32).transpose(1, 0, 2).reshape(D, EH)
    W1a[D] = np.asarray(b1, np.float32).reshape(EH)
    W2a = np.concatenate([np.asarray(W2, np.float32).reshape(EH, O),
                          np.asarray(b2, np.float32)], 0).astype(BF)
    SEL = np.zeros((E, EH), BF)
    for cidx in range(EH):
        SEL[cidx // H, cidx] = 1.0
    consts = {
        "Wg": Wg, "W1a": W1a.astype(BF), "W2a": W2a, "SEL": SEL,
        "I16": np.eye(E, dtype=np.float32),
        "I128": np.eye(128, dtype=BF),
    }
    return xA, consts


def kernel(x, gate_W, gate_b, W1, b1, W2, b2, _loop_reps=1):
    if _loop_reps not in _CACHED:
        _CACHED[_loop_reps] = _build_program(_loop_reps)
    nc = _CACHED[_loop_reps]
    xA, consts = _host_prep(x, gate_W, gate_b, W1, b1, W2, b2)
    in_maps = []
    for cidx in range(NCORES):
        m = dict(consts)
        m["xA"] = xA[cidx]
        in_maps.append(m)
    res = run_bass_kernel_spmd(nc, in_maps, list(range(NCORES)))
    outT = np.concatenate([res.results[i]["out"] for i in range(NCORES)], 1)
    return np.ascontiguousarray(outT.T).astype(np.float32)
